# revision 43
# baseline (speedup 1.0000x reference)
"""Trainium2 Bass kernel for masked edge-softmax attention aggregation.

  score = inputs @ H_v                        [N]
  E[i,j] = exp(adj[i,j]*score[j]) if adj[i,j]!=0 else 0
  out    = (E @ inputs) / rowsum(E)

Sharding: rows of adj over 8 cores (1250 rows each); inputs/H_v replicated,
no collectives. Each core runs the same Tile program on its row shard.

Per-core pipeline (i-quads of up to 4 row-tiles x j-chunks of ~10 blocks):
  DMA adj chunk-slabs [128, ~1280] f32 (contiguous ~5KB/partition runs)
  PE:  4x transpose -> adjT group [128j, 512i] in PSUM
  ACT: E0 = Exp(score_j * adjT)   one op FD=512, per-partition vector scale,
       bf16 out (no row-max subtraction needed: |logit| <= ~9)
  mask m in {0,1} as bf16, split across engines to balance:
       DVE tensor_scalar  m = min(adjT*1e30, 1)  (frac MASK_DVE_FRAC)
       ACT Sign(adjT)                            (rest; adj >= 0)
  apply via the identity E = E0 + m - 1 folded into the accumulation:
       frac APPLY_DVE_FRAC:  e = e0 + m on DVE, 1 matmul per segment
       rest:                 2 matmuls per segment (e0 and m)
  PE:  acc_i[0:ri, 0:129] += seg.T @ [inputs | 1]  (bf16, fused rowsum col)
  fixup per row-tile: sub S_total (= colsum of aug, cancels the "-1"),
       reciprocal of rowsum col, scale, DMA out.
"""

import numpy as np

import concourse.bacc as bacc
import concourse.bass as bass
import concourse.mybir as mybir
import concourse.tile as tile
from concourse.bass_utils import run_bass_kernel_spmd

N = 10000
D = 128
NCORES = 8
R = N // NCORES          # 1250 rows per core
P = 128
NJ = (N + P - 1) // P    # 79 j-blocks, last has 16
NI = (R + P - 1) // P    # 10 i-blocks, last has 122
W = D + 1                # aug width (inputs | ones)

F32 = mybir.dt.float32
F32R = mybir.dt.float32r
BF16 = mybir.dt.bfloat16
AF = mybir.ActivationFunctionType
ALU = mybir.AluOpType

USE_F32R_T = False       # float32r needs producer-side rounding; disabled
import os
MASK_DVE_FRAC = float(os.environ.get("MASK_DVE_FRAC", "0.7"))
APPLY_DVE_FRAC = float(os.environ.get("APPLY_DVE_FRAC", "0.9"))


def _mask_on_dve(b):
    return (b * 7) % 10 < 10 * MASK_DVE_FRAC


def _apply_on_dve(b):
    return (b * 3) % 10 < 10 * APPLY_DVE_FRAC

QUADS = [(0, 4), (4, 4), (8, 2)]           # (first i-tile, count)
QUARTERS = [(0, 10), (10, 10), (20, 10), (30, 10), (40, 10), (50, 10), (60, 10), (70, 9)]  # (first j-block, count)


def _pb(b):
    return P if b < NJ - 1 else N - (NJ - 1) * P


def _ri(i):
    return P if i < NI - 1 else R - (NI - 1) * P


def pt_f32(pt, pb, fd):
    ap = pt[0:pb, 0:fd]
    return ap.bitcast(F32) if ap.dtype == F32R else ap


def build_nc():
    nc = bacc.Bacc("TRN2", target_bir_lowering=False, debug=False, num_devices=NCORES)

    adj_s = nc.dram_tensor("adj_shard", [R, N], F32, kind="ExternalInput")
    inp = nc.dram_tensor("inputs", [N, D], F32, kind="ExternalInput")
    hvb = nc.dram_tensor("hv_bcast", [P, D], F32, kind="ExternalInput")
    ident = nc.dram_tensor("identity", [P, P], F32, kind="ExternalInput")
    out_s = nc.dram_tensor("out_shard", [R, D], F32, kind="ExternalOutput")

    tdt = F32R if USE_F32R_T else F32

    with tile.TileContext(nc) as tc:
        with (
            tc.tile_pool(name="const", bufs=1) as constp,
            tc.tile_pool(name="slab", bufs=4) as slabp,
            tc.tile_pool(name="work", bufs=6) as workp,
            tc.tile_pool(name="fix", bufs=2) as fixp,
            tc.tile_pool(name="psumt", bufs=4, space="PSUM") as psumtp,
            tc.tile_pool(name="psumacc", bufs=1, space="PSUM") as psumaccp,
        ):
            def load_quarter(i0, G, b0, nb):
                # one DMA for the whole quad x j-chunk: the quad's rows are
                # contiguous in HBM, so a 3D AP folds G tiles into one
                # transfer (fewer per-DMA overheads on the serialized stream)
                c0 = b0 * P
                cw = sum(_pb(b0 + bb) for bb in range(nb))
                rows = sum(_ri(i0 + it) for it in range(G))
                qs = slabp.tile([P, G * 1280], F32, tag="qslab", name="qs")
                if rows == G * P:
                    nc.sync.dma_start(
                        qs[:, 0 : G * cw]
                        .rearrange("p (it c) -> p it c", c=cw),
                        adj_s[i0 * P : i0 * P + G * P, c0 : c0 + cw]
                        .rearrange("(it p) c -> p it c", p=P),
                    )
                else:
                    # ragged last quad: full tiles in one DMA + partial tile
                    nfull = rows // P
                    if nfull:
                        nc.sync.dma_start(
                            qs[:, 0 : nfull * cw]
                            .rearrange("p (it c) -> p it c", c=cw),
                            adj_s[i0 * P : (i0 + nfull) * P, c0 : c0 + cw]
                            .rearrange("(it p) c -> p it c", p=P),
                        )
                    rpart = rows - nfull * P
                    nc.sync.dma_start(
                        qs[0:rpart, nfull * cw : nfull * cw + cw],
                        adj_s[
                            (i0 + nfull) * P : (i0 + nfull) * P + rpart,
                            c0 : c0 + cw,
                        ],
                    )
                return [(qs, it * cw, cw) for it in range(G)]

            prefetched = {}

            # ---------------- prologue ----------------
            ident_sb = constp.tile([P, P], F32)
            nc.sync.dma_start(ident_sb[:, :], ident[:, :])
            hv_sb = constp.tile([P, D], F32)
            nc.sync.dma_start(hv_sb[:, :], hvb[:, :])

            # inputs staged as [p, b*D + d] = inputs[b*128+p, d]
            inp_sb = constp.tile([P, NJ * D], F32)
            main_rows = (NJ - 1) * P  # 9984

            def load_inp_chunk(c0, nb):
                nc.sync.dma_start(
                    inp_sb[:, c0 * D : (c0 + nb) * D].rearrange(
                        "p (b d) -> p b d", d=D
                    ),
                    inp[c0 * P : (c0 + nb) * P, :].rearrange(
                        "(b p) d -> p b d", p=P
                    ),
                )

            for c0, nb in ((0, 20), (20, 20), (40, 20), (60, NJ - 1 - 60)):
                load_inp_chunk(c0, nb)
            nc.vector.memset(inp_sb[:, (NJ - 1) * D : NJ * D], 0.0)
            nc.sync.dma_start(
                inp_sb[0 : _pb(NJ - 1), (NJ - 1) * D : NJ * D],
                inp[main_rows:N, :],
            )

            # score[p, b] = sum_d inputs[b*128+p, d] * H_v[d]
            # wide chunked TT+reduce; tail rows of inp_sb are zeroed so the
            # partial last block just yields score 0 for unused partitions
            score_sb = constp.tile([P, NJ], F32)
            for c0, nb in ((0, 20), (20, 20), (40, 20), (60, NJ - 60)):
                stmp = workp.tile([P, 20 * D], F32, tag="stmp", bufs=2)
                hv_rep = hv_sb[:, :].rearrange("p (o d) -> p o d", o=1).broadcast_to(
                    [P, nb, D]
                )
                nc.vector.tensor_tensor(
                    stmp[:, 0 : nb * D].rearrange("p (b d) -> p b d", d=D),
                    inp_sb[:, c0 * D : (c0 + nb) * D].rearrange(
                        "p (b d) -> p b d", d=D
                    ),
                    hv_rep,
                    ALU.mult,
                )
                nc.vector.tensor_reduce(
                    score_sb[:, c0 : c0 + nb],
                    stmp[:, 0 : nb * D].rearrange("p (b d) -> p b d", d=D),
                    axis=mybir.AxisListType.X,
                    op=ALU.add,
                )

            # aug = [inputs | 1] in bf16, tiles of width 129 per j-block
            # (zeroed inp_sb tail rows make the partial last block harmless)
            aug_sb = constp.tile([P, NJ * W], BF16)
            aug3 = aug_sb[:, :].rearrange("p (b w) -> p b w", w=W)
            for c0, nb in ((0, 20), (20, 20), (40, 20), (60, NJ - 60)):
                nc.vector.tensor_copy(
                    aug3[:, c0 : c0 + nb, 0:D],
                    inp_sb[:, c0 * D : (c0 + nb) * D].rearrange(
                        "p (b d) -> p b d", d=D
                    ),
                )
                nc.vector.memset(aug3[:, c0 : c0 + nb, D : D + 1], 1.0)



            # ---------------- main loop ----------------
            s_bcast = None
            for i0, G in QUADS:
                FD = G * P
                accs = [
                    psumaccp.tile([P, W], F32, tag=f"acc{it}", name=f"acc{it}")
                    for it in range(G)
                ]
                for b0, nb in QUARTERS:
                    if (i0, b0) in prefetched:
                        qslabs = prefetched.pop((i0, b0))
                    else:
                        qslabs = load_quarter(i0, G, b0, nb)
                    for bb in range(nb):
                        b = b0 + bb
                        pb = _pb(b)
                        pt = psumtp.tile([P, 512], tdt, tag="pt")
                        for it in range(G):
                            ri = _ri(i0 + it)
                            qs, qoff, qcw = qslabs[it]
                            in_ap = qs[0:ri, qoff + bb * P : qoff + bb * P + pb]
                            id_ap = ident_sb[0:ri, 0:ri]
                            if USE_F32R_T:
                                in_ap = in_ap.bitcast(F32R)
                                id_ap = id_ap.bitcast(F32R)
                            nc.tensor.transpose(
                                pt[0:pb, it * P : it * P + ri], in_ap, id_ap
                            )
                        e0 = workp.tile([P, 512], BF16, tag="e0")
                        nc.scalar.activation(
                            e0[0:pb, 0:FD],
                            pt_f32(pt, pb, FD),
                            AF.Exp,
                            bias=0.0,
                            scale=score_sb[0:pb, b : b + 1],
                        )
                        m = workp.tile([P, 512], BF16, tag="m")
                        if _mask_on_dve(b):
                            nc.vector.tensor_scalar(
                                m[0:pb, 0:FD], pt_f32(pt, pb, FD), 1e30, 1.0,
                                ALU.mult, ALU.min,
                            )
                        else:
                            nc.scalar.activation(m[0:pb, 0:FD], pt_f32(pt, pb, FD), AF.Sign)
                        rhs = aug_sb[0:pb, b * W : (b + 1) * W]
                        if _apply_on_dve(b):
                            # e = e0 + m: same contribution as the 2-MM
                            # path; the -1 is folded into the S_total fixup
                            e = workp.tile([P, 512], BF16, tag="e")
                            nc.vector.tensor_add(
                                e[0:pb, 0:FD], e0[0:pb, 0:FD], m[0:pb, 0:FD]
                            )
                            for it in range(G):
                                ri = _ri(i0 + it)
                                nc.tensor.matmul(
                                    accs[it][0:ri, :],
                                    e[0:pb, it * P : it * P + ri],
                                    rhs,
                                    start=(b == 0),
                                    stop=(b == NJ - 1),
                                )
                        else:
                            for it in range(G):
                                ri = _ri(i0 + it)
                                nc.tensor.matmul(
                                    accs[it][0:ri, :],
                                    e0[0:pb, it * P : it * P + ri],
                                    rhs,
                                    start=(b == 0),
                                    stop=False,
                                )
                                nc.tensor.matmul(
                                    accs[it][0:ri, :],
                                    m[0:pb, it * P : it * P + ri],
                                    rhs,
                                    start=False,
                                    stop=(b == NJ - 1),
                                )
                if s_bcast is None:
                    # S_total[d] = colsum of aug (for E = E0 + m - 1: acc
                    # holds (E0+m)@aug; fixup subtracts S_total = 1@aug).
                    # Emitted after the first quad so the 80 small matmuls
                    # don't head-of-line-block the first transposes in the
                    # in-order PE queue.
                    ones_sb = constp.tile([P, 1], BF16)
                    nc.vector.memset(ones_sb[:, :], 1.0)
                    psum_s = psumtp.tile([P, 512], F32, tag="pt", name="psum_s")
                    for b in range(NJ):
                        pb = _pb(b)
                        nc.tensor.matmul(
                            psum_s[0:1, 0:W],
                            ones_sb[0:pb, :],
                            aug_sb[0:pb, b * W : (b + 1) * W],
                            start=(b == 0),
                            stop=(b == NJ - 1),
                        )
                    s_row = constp.tile([1, W], F32)
                    nc.vector.tensor_copy(s_row[0:1, :], psum_s[0:1, 0:W])
                    ones_row = constp.tile([1, P], F32)
                    nc.vector.memset(ones_row[0:1, :], 1.0)
                    psum_b = psumtp.tile([P, 512], F32, tag="pt", name="psum_b")
                    nc.tensor.matmul(
                        psum_b[:, 0:W], ones_row[0:1, :], s_row[0:1, :],
                        start=True, stop=True,
                    )
                    s_bcast = constp.tile([P, W], F32)
                    nc.vector.tensor_copy(s_bcast[:, :], psum_b[:, 0:W])

                # normalize rows by the fused rowsum column
                for it in range(G):
                    ri = _ri(i0 + it)
                    tmpo = fixp.tile([P, W], F32, tag="tmpo")
                    nc.vector.tensor_sub(
                        tmpo[0:ri, :], accs[it][0:ri, :], s_bcast[0:ri, :]
                    )
                    rec = fixp.tile([P, 1], F32, tag="rec")
                    nc.vector.reciprocal(rec[0:ri, :], tmpo[0:ri, D : D + 1])
                    osb = fixp.tile([P, D], F32, tag="osb")
                    nc.vector.tensor_scalar(
                        osb[0:ri, :], tmpo[0:ri, 0:D], rec[0:ri, :], None, ALU.mult
                    )
                    nc.sync.dma_start(
                        out_s[(i0 + it) * P : (i0 + it) * P + ri, :], osb[0:ri, :]
                    )

    nc.compile()
    return nc


_NC = None


def _get_nc():
    global _NC
    if _NC is None:
        _NC = build_nc()
    return _NC


def kernel(inputs, adj, H_v, _trace=False, _trace_kwargs=None):
    inputs = np.ascontiguousarray(np.asarray(inputs), dtype=np.float32)
    adj = np.ascontiguousarray(np.asarray(adj), dtype=np.float32)
    H_v = np.asarray(H_v, dtype=np.float32)

    nc = _get_nc()
    hv_bcast = np.ascontiguousarray(np.tile(H_v.reshape(1, D), (P, 1)))
    identity = np.eye(P, dtype=np.float32)
    in_maps = [
        {
            "adj_shard": np.ascontiguousarray(adj[c * R : (c + 1) * R, :]),
            "inputs": inputs,
            "hv_bcast": hv_bcast,
            "identity": identity,
        }
        for c in range(NCORES)
    ]
    kw = {}
    if _trace:
        kw = dict(trace=True, **(_trace_kwargs or {}))
    res = run_bass_kernel_spmd(nc, in_maps, list(range(NCORES)), **kw)
    if _trace:
        kernel._last_results = res
    outs = res.results
    return np.concatenate(
        [np.asarray(outs[c]["out_shard"], dtype=np.float32) for c in range(NCORES)],
        axis=0,
    )
